# revision 10
# baseline (speedup 1.0000x reference)
import numpy as np
import concourse.bass as bass
import concourse.bacc as bacc
import concourse.mybir as mybir
from concourse import tile
from concourse.bass_utils import run_bass_kernel_spmd

NX = 2048
NY = 2048
NCORES = 8
R = NX // NCORES
SLAB = R + 2
YP = NY + 2
TB = [0, 130]
W = 512
NCH = NY // W

TAU = 0.6
INV_TAU = 1.0 / TAU
FCOEF = 1.0 - INV_TAU
W1P = INV_TAU * (1.0 / 9.0)
W5P = INV_TAU * (1.0 / 36.0)
W0P = INV_TAU * (4.0 / 9.0)

EX = [0, 1, 0, -1, 0, 1, -1, -1, 1]
EY = [0, 0, 1, 0, -1, 1, 1, -1, -1]
OPP = [0, 3, 4, 1, 2, 7, 8, 5, 6]

FXR0 = 126
FXNR = 6
FXSEG = 8
FXW = NY // FXSEG
FXF = FXW + 4
FXP = FXSEG * FXNR

FP32 = mybir.dt.float32
BF16 = mybir.dt.bfloat16
U8 = mybir.dt.uint8
AL = mybir.AluOpType

def _pamaoa(ey):
    pa = 1 + ey
    return pa, pa, 1


def _build_program():
    nc = bacc.Bacc(None)

    fin_d = nc.declare_dram_parameter("fin", [12, SLAB, YP], BF16, isOutput=False)
    mk_d = nc.declare_dram_parameter("mk", [3, SLAB, YP], U8, isOutput=False)
    wts_d = nc.declare_dram_parameter("wts", [128, 4 * 128], BF16, isOutput=False)
    pfin_d = nc.declare_dram_parameter("pfin", [12, FXP, FXF], BF16, isOutput=False)
    pmk_d = nc.declare_dram_parameter("pmk", [3, FXP, FXF], U8, isOutput=False)
    pwts_d = nc.declare_dram_parameter("pwts", [FXP, 4 * FXP], BF16, isOutput=False)
    out_d = nc.declare_dram_parameter("out", [12, SLAB, NY], BF16, isOutput=True)

    def tt(eng, o, a, b, op):
        eng.tensor_tensor(o, a, b, op)

    with tile.TileContext(nc) as tc, tc.tile_pool(name="cst", bufs=1) as cst:
        wts = cst.tile([128, 4 * 128], BF16)
        pwts = cst.tile([FXP, 4 * FXP], BF16)
        nc.sync.dma_start(out=wts[:], in_=wts_d[:, :])
        nc.sync.dma_start(out=pwts[:], in_=pwts_d[:, :])
        def wblk(s):
            return wts[:, (s + 1) * 128:(s + 2) * 128]
        def pwblk(s):
            return pwts[:, (s + 1) * FXP:(s + 2) * FXP]

        with (
            tc.tile_pool(name="io", bufs=2) as io,
            tc.tile_pool(name="mki", bufs=1) as mki,
            tc.tile_pool(name="o2", bufs=1) as o2p,
            tc.tile_pool(name="psS", bufs=1, space="PSUM") as psS,
            tc.tile_pool(name="scr", bufs=1) as scr,
        ):
            for tb in TB:
                IN = io.tile([128, 12 * YP], BF16, tag="IN")
                MK = mki.tile([128, 3 * YP], U8, tag="MK")
                OUT2 = o2p.tile([128, 3 * NY], BF16, tag="OUT2")
                nc.sync.dma_start(
                    out=IN[:].rearrange("p (c y) -> p c y", c=12),
                    in_=fin_d[:, tb:tb + 128, :].rearrange("c p y -> p c y"))
                nc.sync.dma_start(
                    out=MK[:].rearrange("p (c y) -> p c y", c=3),
                    in_=mk_d[:, tb:tb + 128, :].rearrange("c p y -> p c y"))

                def F(i, a=0, b=YP):
                    return IN[:, i * YP + a:i * YP + b]
                UX = F(9); UY = F(10); RH = F(11)
                def MKV(s, a, b):
                    return MK[:, (s + 1) * YP + a:(s + 1) * YP + b]

                def S(name, dt=BF16, wdt=YP):
                    return scr.tile([128, wdt], dt, tag=name, name=name)

                r1 = S("r1"); r2 = S("r2"); t1 = S("t1"); t2 = S("t2")
                sv = S("sv"); dv = S("dv"); rs = S("rs"); rd = S("rd")
                a5 = S("a5"); a6 = S("a6"); uq = S("uq")
                V = nc.vector; P = nc.gpsimd

                tt(P, r1[:], RH, UX, AL.mult)
                tt(P, r2[:], RH, UY, AL.mult)
                tt(P, t1[:], UX, r1[:], AL.mult)
                tt(P, t2[:], UY, r2[:], AL.mult)
                tt(V, uq[:], t1[:], t2[:], AL.add)
                V.tensor_scalar_mul(uq[:], uq[:], -1.5)
                tt(V, uq[:], uq[:], RH, AL.add)
                tt(P, sv[:], UX, UY, AL.add)
                tt(P, dv[:], UX, UY, AL.subtract)
                tt(V, rs[:], r1[:], r2[:], AL.add)
                tt(V, rd[:], r1[:], r2[:], AL.subtract)
                tt(P, a5[:], sv[:], rs[:], AL.mult)
                tt(P, a6[:], dv[:], rd[:], AL.mult)
                V.tensor_scalar_mul(t1[:], t1[:], 4.5 * W1P)
                V.tensor_scalar_mul(t2[:], t2[:], 4.5 * W1P)
                V.tensor_scalar_mul(a5[:], a5[:], 4.5 * W5P)
                V.tensor_scalar_mul(a6[:], a6[:], 4.5 * W5P)
                V.tensor_scalar_mul(r1[:], r1[:], 3.0 * W1P)
                V.tensor_scalar_mul(r2[:], r2[:], 3.0 * W1P)
                V.tensor_scalar_mul(rs[:], rs[:], 3.0 * W5P)
                V.tensor_scalar_mul(rd[:], rd[:], 3.0 * W5P)
                V.tensor_scalar_mul(sv[:], uq[:], W1P)
                V.tensor_scalar_mul(dv[:], uq[:], W5P)
                V.tensor_scalar_mul(uq[:], uq[:], W0P)
                tt(P, t1[:], t1[:], sv[:], AL.add)
                tt(P, t2[:], t2[:], sv[:], AL.add)
                tt(P, a5[:], a5[:], dv[:], AL.add)
                tt(P, a6[:], a6[:], dv[:], AL.add)
                tt(V, F(0), F(0), uq[:], AL.add)
                tt(V, F(1), F(1), t1[:], AL.add)
                tt(V, F(1), F(1), r1[:], AL.add)
                tt(V, F(3), F(3), t1[:], AL.add)
                tt(V, F(3), F(3), r1[:], AL.subtract)
                tt(V, F(2), F(2), t2[:], AL.add)
                tt(V, F(2), F(2), r2[:], AL.add)
                tt(P, F(4), F(4), t2[:], AL.add)
                tt(P, F(4), F(4), r2[:], AL.subtract)
                tt(P, F(5), F(5), a5[:], AL.add)
                tt(P, F(5), F(5), rs[:], AL.add)
                tt(P, F(7), F(7), a5[:], AL.add)
                tt(P, F(7), F(7), rs[:], AL.subtract)
                tt(P, F(6), F(6), a6[:], AL.add)
                tt(P, F(6), F(6), rd[:], AL.subtract)
                tt(P, F(8), F(8), a6[:], AL.add)
                tt(P, F(8), F(8), rd[:], AL.add)

                for i, j in ((1, 3), (2, 4), (5, 7), (6, 8)):
                    bbs = {}
                    for d, tag in ((i, "bb0"), (j, "bb1")):
                        exd, eyd = EX[d], EY[d]
                        pa, ma, oa = _pamaoa(eyd)
                        bb = S(tag, wdt=NY)
                        for c in range(NCH):
                            sp = psS.tile([128, W], FP32, tag=f"sp{c % 4}",
                                          name=f"sp{d}_{c}")
                            nc.tensor.matmul(sp[:], wblk(exd),
                                             F(OPP[d], ma + W * c, ma + W * (c + 1)))
                            nc.scalar.copy(bb[:, W * c:W * (c + 1)], sp[:])
                        bbs[d] = bb
                    for d in (i, j):
                        exd, eyd = EX[d], EY[d]
                        pa, ma, oa = _pamaoa(eyd)
                        V.copy_predicated(F(d, oa, oa + NY),
                                          MKV(exd, pa, pa + NY), bbs[d][:])

                for i in (2, 5, 6):
                    nc.scalar.copy(F(i, 0, 1), F(i, NY, NY + 1))
                for i in (4, 7, 8):
                    nc.scalar.copy(F(i, NY + 1, NY + 2), F(i, 1, 2))

                RA = S("r1"); RB = S("r2"); R0 = S("t1")
                MA_ = S("t2"); M0 = S("a5"); MB = S("a6")
                tt(P, RA[:, 0:NY], F(1, 1, 1 + NY), F(5, 0, NY), AL.add)
                tt(P, RA[:, 0:NY], RA[:, 0:NY], F(8, 2, 2 + NY), AL.add)
                tt(P, RB[:, 0:NY], F(3, 1, 1 + NY), F(6, 0, NY), AL.add)
                tt(P, RB[:, 0:NY], RB[:, 0:NY], F(7, 2, 2 + NY), AL.add)
                tt(V, R0[:, 0:NY], F(0, 1, 1 + NY), F(2, 0, NY), AL.add)
                tt(V, R0[:, 0:NY], R0[:, 0:NY], F(4, 2, 2 + NY), AL.add)
                tt(V, MA_[:, 0:NY], F(5, 0, NY), F(8, 2, 2 + NY), AL.subtract)
                tt(P, M0[:, 0:NY], F(2, 0, NY), F(4, 2, 2 + NY), AL.subtract)
                tt(P, MB[:, 0:NY], F(6, 0, NY), F(7, 2, 2 + NY), AL.subtract)

                sm1 = S("sv", wdt=NY); sm2 = S("dv", wdt=NY)
                inv = S("inv", FP32, wdt=NY)
                for c in range(NCH):
                    cs = slice(W * c, W * (c + 1))
                    rp = psS.tile([128, W], FP32, tag="sp0", name=f"rp{c}")
                    nc.tensor.matmul(rp[:], wblk(-1), RA[:, cs], start=True, stop=False)
                    nc.tensor.matmul(rp[:], wblk(0), R0[:, cs], start=False, stop=False)
                    nc.tensor.matmul(rp[:], wblk(1), RB[:, cs], start=False, stop=True)
                    m1p = psS.tile([128, W], FP32, tag="sp1", name=f"m1p{c}")
                    nc.tensor.matmul(m1p[:], wblk(-1), RA[:, cs], start=True, stop=False)
                    nc.tensor.matmul(m1p[:], wts[:, 3 * 128:4 * 128], RB[:, cs],
                                     start=False, stop=True)
                    m2p = psS.tile([128, W], FP32, tag="sp2", name=f"m2p{c}")
                    nc.tensor.matmul(m2p[:], wblk(-1), MA_[:, cs], start=True, stop=False)
                    nc.tensor.matmul(m2p[:], wblk(0), M0[:, cs], start=False, stop=False)
                    nc.tensor.matmul(m2p[:], wblk(1), MB[:, cs], start=False, stop=True)
                    nc.scalar.copy(OUT2[:, cs], rp[:])
                    V.reciprocal_approx_fast(inv[:, cs], rp[:])
                    nc.scalar.copy(sm1[:, cs], m1p[:])
                    nc.scalar.copy(sm2[:, cs], m2p[:])
                invb = S("invb", wdt=NY)
                nc.scalar.copy(invb[:], inv[:])
                tt(V, OUT2[:, NY:2 * NY], sm1[:], invb[:], AL.mult)
                tt(V, OUT2[:, 2 * NY:3 * NY], sm2[:], invb[:], AL.mult)

                nc.sync.dma_start(
                    out=out_d[0:9, tb:tb + 128, :].rearrange("c p y -> p c y"),
                    in_=IN[:].rearrange("p (c y) -> p c y", c=12)[:, 0:9, 1:1 + NY])
                nc.sync.dma_start(
                    out=out_d[9:12, tb + 1:tb + 127, :].rearrange("c p y -> p c y"),
                    in_=OUT2[1:127, :].rearrange("p (c y) -> p c y", c=3))

            pIN = mki.tile([FXP, 12 * FXF], BF16, tag="pIN")
            pMK = mki.tile([FXP, 3 * FXF], U8, tag="pMK")
            nc.sync.dma_start(
                out=pIN[:].rearrange("p (c y) -> p c y", c=12),
                in_=pfin_d[:, :, :].rearrange("c p y -> p c y"))
            nc.sync.dma_start(
                out=pMK[:].rearrange("p (c y) -> p c y", c=3),
                in_=pmk_d[:, :, :].rearrange("c p y -> p c y"))

            def pF(i, a=0, b=FXF):
                return pIN[:, i * FXF + a:i * FXF + b]
            pUX = pF(9); pUY = pF(10); pRH = pF(11)
            def pMKV(s, a, b):
                return pMK[:, (s + 1) * FXF + a:(s + 1) * FXF + b]

            def PS(name, dt=BF16, wdt=FXF):
                return scr.tile([FXP, wdt], dt, tag=f"p_{name}", name=f"p_{name}")

            r1 = PS("r1"); r2 = PS("r2"); t1 = PS("t1"); t2 = PS("t2")
            sv = PS("sv"); dv = PS("dv"); rs = PS("rs"); rd = PS("rd")
            a5 = PS("a5"); a6 = PS("a6"); uq = PS("uq")
            V = nc.vector; P = nc.gpsimd

            tt(P, r1[:], pRH, pUX, AL.mult)
            tt(P, r2[:], pRH, pUY, AL.mult)
            tt(P, t1[:], pUX, r1[:], AL.mult)
            tt(P, t2[:], pUY, r2[:], AL.mult)
            tt(V, uq[:], t1[:], t2[:], AL.add)
            V.tensor_scalar_mul(uq[:], uq[:], -1.5)
            tt(V, uq[:], uq[:], pRH, AL.add)
            tt(P, sv[:], pUX, pUY, AL.add)
            tt(P, dv[:], pUX, pUY, AL.subtract)
            tt(V, rs[:], r1[:], r2[:], AL.add)
            tt(V, rd[:], r1[:], r2[:], AL.subtract)
            tt(P, a5[:], sv[:], rs[:], AL.mult)
            tt(P, a6[:], dv[:], rd[:], AL.mult)
            V.tensor_scalar_mul(t1[:], t1[:], 4.5 * W1P)
            V.tensor_scalar_mul(t2[:], t2[:], 4.5 * W1P)
            V.tensor_scalar_mul(a5[:], a5[:], 4.5 * W5P)
            V.tensor_scalar_mul(a6[:], a6[:], 4.5 * W5P)
            V.tensor_scalar_mul(r1[:], r1[:], 3.0 * W1P)
            V.tensor_scalar_mul(r2[:], r2[:], 3.0 * W1P)
            V.tensor_scalar_mul(rs[:], rs[:], 3.0 * W5P)
            V.tensor_scalar_mul(rd[:], rd[:], 3.0 * W5P)
            V.tensor_scalar_mul(sv[:], uq[:], W1P)
            V.tensor_scalar_mul(dv[:], uq[:], W5P)
            V.tensor_scalar_mul(uq[:], uq[:], W0P)
            tt(P, t1[:], t1[:], sv[:], AL.add)
            tt(P, t2[:], t2[:], sv[:], AL.add)
            tt(P, a5[:], a5[:], dv[:], AL.add)
            tt(P, a6[:], a6[:], dv[:], AL.add)
            tt(V, pF(0), pF(0), uq[:], AL.add)
            tt(V, pF(1), pF(1), t1[:], AL.add)
            tt(V, pF(1), pF(1), r1[:], AL.add)
            tt(V, pF(3), pF(3), t1[:], AL.add)
            tt(V, pF(3), pF(3), r1[:], AL.subtract)
            tt(V, pF(2), pF(2), t2[:], AL.add)
            tt(V, pF(2), pF(2), r2[:], AL.add)
            tt(P, pF(4), pF(4), t2[:], AL.add)
            tt(P, pF(4), pF(4), r2[:], AL.subtract)
            tt(P, pF(5), pF(5), a5[:], AL.add)
            tt(P, pF(5), pF(5), rs[:], AL.add)
            tt(P, pF(7), pF(7), a5[:], AL.add)
            tt(P, pF(7), pF(7), rs[:], AL.subtract)
            tt(P, pF(6), pF(6), a6[:], AL.add)
            tt(P, pF(6), pF(6), rd[:], AL.subtract)
            tt(P, pF(8), pF(8), a6[:], AL.add)
            tt(P, pF(8), pF(8), rd[:], AL.add)

            for i, j in ((1, 3), (2, 4), (5, 7), (6, 8)):
                bbs = {}
                for d, tag in ((i, "bb0"), (j, "bb1")):
                    exd, eyd = EX[d], EY[d]
                    pa, ma, oa = _pamaoa(eyd)
                    bb = PS(tag, wdt=FXW + 2)
                    sp = psS.tile([FXP, FXW + 2], FP32, tag=f"psp{0 if d == i else 1}",
                                  name=f"psp{d}")
                    nc.tensor.matmul(sp[:], pwblk(exd), pF(OPP[d], ma, ma + FXW + 2))
                    nc.scalar.copy(bb[:], sp[:])
                    bbs[d] = bb
                for d in (i, j):
                    exd, eyd = EX[d], EY[d]
                    pa, ma, oa = _pamaoa(eyd)
                    V.copy_predicated(pF(d, oa, oa + FXW + 2),
                                      pMKV(exd, pa, pa + FXW + 2), bbs[d][:])

            RA = PS("r1"); RB = PS("r2"); R0 = PS("t1")
            MA_ = PS("t2"); M0 = PS("a5"); MB = PS("a6")
            tt(P, RA[:, 0:FXW], pF(1, 2, 2 + FXW), pF(5, 1, 1 + FXW), AL.add)
            tt(P, RA[:, 0:FXW], RA[:, 0:FXW], pF(8, 3, 3 + FXW), AL.add)
            tt(P, RB[:, 0:FXW], pF(3, 2, 2 + FXW), pF(6, 1, 1 + FXW), AL.add)
            tt(P, RB[:, 0:FXW], RB[:, 0:FXW], pF(7, 3, 3 + FXW), AL.add)
            tt(V, R0[:, 0:FXW], pF(0, 2, 2 + FXW), pF(2, 1, 1 + FXW), AL.add)
            tt(V, R0[:, 0:FXW], R0[:, 0:FXW], pF(4, 3, 3 + FXW), AL.add)
            tt(V, MA_[:, 0:FXW], pF(5, 1, 1 + FXW), pF(8, 3, 3 + FXW), AL.subtract)
            tt(P, M0[:, 0:FXW], pF(2, 1, 1 + FXW), pF(4, 3, 3 + FXW), AL.subtract)
            tt(P, MB[:, 0:FXW], pF(6, 1, 1 + FXW), pF(7, 3, 3 + FXW), AL.subtract)

            rp = psS.tile([FXP, FXW], FP32, tag="psp0", name="prp")
            nc.tensor.matmul(rp[:], pwblk(-1), RA[:, 0:FXW], start=True, stop=False)
            nc.tensor.matmul(rp[:], pwblk(0), R0[:, 0:FXW], start=False, stop=False)
            nc.tensor.matmul(rp[:], pwblk(1), RB[:, 0:FXW], start=False, stop=True)
            m1p = psS.tile([FXP, FXW], FP32, tag="psp1", name="pm1p")
            nc.tensor.matmul(m1p[:], pwblk(-1), RA[:, 0:FXW], start=True, stop=False)
            nc.tensor.matmul(m1p[:], pwts[:, 3 * FXP:4 * FXP], RB[:, 0:FXW],
                             start=False, stop=True)
            m2p = psS.tile([FXP, FXW], FP32, tag="psp2", name="pm2p")
            nc.tensor.matmul(m2p[:], pwblk(-1), MA_[:, 0:FXW], start=True, stop=False)
            nc.tensor.matmul(m2p[:], pwblk(0), M0[:, 0:FXW], start=False, stop=False)
            nc.tensor.matmul(m2p[:], pwblk(1), MB[:, 0:FXW], start=False, stop=True)
            pinv = PS("pinv", FP32, wdt=FXW)
            psm1 = PS("sv", wdt=FXW); psm2 = PS("dv", wdt=FXW)
            nc.scalar.copy(pF(9, 2, 2 + FXW), rp[:])
            V.reciprocal_approx_fast(pinv[:], rp[:])
            nc.scalar.copy(psm1[:], m1p[:])
            nc.scalar.copy(psm2[:], m2p[:])
            pinvb = PS("pinvb", wdt=FXW)
            nc.scalar.copy(pinvb[:], pinv[:])
            tt(V, pF(10, 2, 2 + FXW), psm1[:], pinvb[:], AL.mult)
            tt(V, pF(11, 2, 2 + FXW), psm2[:], pinvb[:], AL.mult)

            for sg in range(FXSEG):
                nc.sync.dma_start(
                    out=out_d[:, 127:131, sg * FXW:(sg + 1) * FXW].rearrange(
                        "c r y -> r c y"),
                    in_=pIN[sg * FXNR + 1:sg * FXNR + 5, :].rearrange(
                        "p (c y) -> p c y", c=12)[:, :, 2:2 + FXW])

    nc.finalize()
    return nc


_NC_CACHE = None


def _get_nc():
    global _NC_CACHE
    if _NC_CACHE is None:
        _NC_CACHE = _build_program()
    return _NC_CACHE


def _wts_np():
    import ml_dtypes
    m = np.zeros((128, 4 * 128), np.float32)
    for s in (-1, 0, 1):
        for q in range(128):
            k = q + s
            if 0 <= k < 128:
                m[k, (s + 1) * 128 + q] = 1.0
    for q in range(128):
        k = q + 1
        if 0 <= k < 128:
            m[k, 3 * 128 + q] = -1.0
    return m.astype(ml_dtypes.bfloat16)


def _pwts_np():
    import ml_dtypes
    m = np.zeros((FXP, 4 * FXP), np.float32)
    for s in (-1, 0, 1):
        for sg in range(FXSEG):
            for j in range(FXNR):
                q = sg * FXNR + j
                jk = j + s
                if 0 <= jk < FXNR:
                    m[sg * FXNR + jk, (s + 1) * FXP + q] = 1.0
    for sg in range(FXSEG):
        for j in range(FXNR):
            q = sg * FXNR + j
            jk = j + 1
            if 0 <= jk < FXNR:
                m[sg * FXNR + jk, 3 * FXP + q] = -1.0
    return m.astype(ml_dtypes.bfloat16)


def _prep_inputs(f, rho, u, obstacle_mask):
    import ml_dtypes
    f = np.asarray(f, dtype=np.float32)
    rho = np.asarray(rho, dtype=np.float32)
    u = np.asarray(u, dtype=np.float32)
    mask = np.asarray(obstacle_mask).astype(np.uint8)

    planes = np.empty((12, NX, NY), np.float32)
    for i in range(9):
        planes[i] = FCOEF * f[..., i]
    planes[9] = u[..., 0]
    planes[10] = u[..., 1]
    planes[11] = rho
    planes_b = planes.astype(ml_dtypes.bfloat16)

    wts = _wts_np()
    pwts = _pwts_np()
    rows_idx = np.arange(-1, R + 1)
    cols_idx = np.arange(-1, NY + 1) % NY
    in_maps = []
    for k in range(NCORES):
        lo = k * R
        ridx = (lo + rows_idx) % NX
        fin = planes_b[:, ridx][:, :, cols_idx]
        mk = np.empty((3, SLAB, YP), np.uint8)
        for si, s in enumerate((-1, 0, 1)):
            mk[si] = mask[(lo + rows_idx + s) % NX][:, cols_idx]
        pfin = np.empty((12, FXP, FXF), ml_dtypes.bfloat16)
        pmk = np.empty((3, FXP, FXF), np.uint8)
        frows = (lo - 1 + FXR0 + np.arange(FXNR)) % NX
        for sg in range(FXSEG):
            ccols = (sg * FXW + np.arange(-2, FXW + 2)) % NY
            seg = planes_b[:, frows][:, :, ccols]
            pfin[:, sg * FXNR:(sg + 1) * FXNR] = seg
            for si, s in enumerate((-1, 0, 1)):
                pmk[si, sg * FXNR:(sg + 1) * FXNR] = \
                    mask[(frows + s) % NX][:, ccols]
        in_maps.append({
            "fin": np.ascontiguousarray(fin),
            "mk": np.ascontiguousarray(mk),
            "wts": wts,
            "pfin": np.ascontiguousarray(pfin),
            "pmk": np.ascontiguousarray(pmk),
            "pwts": pwts,
        })
    return in_maps


def kernel(f, rho, u, obstacle_mask, _trace=False):
    in_maps = _prep_inputs(f, rho, u, obstacle_mask)
    nc = _get_nc()
    res = run_bass_kernel_spmd(nc, in_maps, list(range(NCORES)),
                               trace=bool(_trace))
    full = np.empty((NX, NY, 12), np.float32)
    chan = np.concatenate(
        [np.asarray(res.results[k]["out"])[:, 1:R + 1, :].astype(np.float32)
         for k in range(NCORES)], axis=1)
    for c in range(9):
        full[..., c] = np.roll(chan[c], (EX[c], EY[c]), axis=(0, 1))
    full[..., 9] = chan[9]
    full[..., 10] = chan[10]
    full[..., 11] = chan[11]
    if _trace:
        return full, res
    return full


# revision 14
# speedup vs baseline: 1.1074x; 1.1074x over previous
import numpy as np
import concourse.bass as bass
import concourse.bacc as bacc
import concourse.mybir as mybir
from concourse import tile
from concourse.bass_utils import run_bass_kernel_spmd

NX = 2048
NY = 2048
NCORES = 8
R = NX // NCORES
SLAB = R + 2
YP = NY + 2
TB = [0, 130]
W = 512
NCH = NY // W

TAU = 0.6
INV_TAU = 1.0 / TAU
FCOEF = 1.0 - INV_TAU
W1P = INV_TAU * (1.0 / 9.0)
W5P = INV_TAU * (1.0 / 36.0)
W0P = INV_TAU * (4.0 / 9.0)

EX = [0, 1, 0, -1, 0, 1, -1, -1, 1]
EY = [0, 0, 1, 0, -1, 1, 1, -1, -1]
OPP = [0, 3, 4, 1, 2, 7, 8, 5, 6]

FXR0 = 126
FXNR = 6
FXSEG = 8
FXW = NY // FXSEG
FXF = FXW + 4
FXP = FXSEG * FXNR

FP32 = mybir.dt.float32
BF16 = mybir.dt.bfloat16
U8 = mybir.dt.uint8
AL = mybir.AluOpType

def _pamaoa(ey):
    pa = 1 + ey
    return pa, pa, 1


def _build_program():
    nc = bacc.Bacc(None)

    fin_d = nc.declare_dram_parameter("fin", [12, SLAB, YP], BF16, isOutput=False)
    mk_d = nc.declare_dram_parameter("mk", [3, SLAB, YP], U8, isOutput=False)
    wts_d = nc.declare_dram_parameter("wts", [128, 6 * 128], BF16, isOutput=False)
    pfin_d = nc.declare_dram_parameter("pfin", [12, FXP, FXF], BF16, isOutput=False)
    pmk_d = nc.declare_dram_parameter("pmk", [3, FXP, FXF], U8, isOutput=False)
    pwts_d = nc.declare_dram_parameter("pwts", [FXP, 6 * FXP], BF16, isOutput=False)
    out_d = nc.declare_dram_parameter("out", [12, SLAB, NY], BF16, isOutput=True)

    def tt(eng, o, a, b, op):
        eng.tensor_tensor(o, a, b, op)

    with tile.TileContext(nc) as tc, tc.tile_pool(name="cst", bufs=1) as cst:
        wts = cst.tile([128, 6 * 128], BF16)
        pwts = cst.tile([FXP, 6 * FXP], BF16)
        nc.sync.dma_start(out=wts[:], in_=wts_d[:, :])
        nc.sync.dma_start(out=pwts[:], in_=pwts_d[:, :])
        def wblk(s, neg=False):
            b = (s + 1) + (3 if neg else 0)
            return wts[:, b * 128:(b + 1) * 128]
        def pwblk(s, neg=False):
            b = (s + 1) + (3 if neg else 0)
            return pwts[:, b * FXP:(b + 1) * FXP]

        with (
            tc.tile_pool(name="io", bufs=2) as io,
            tc.tile_pool(name="mki", bufs=1) as mki,
            tc.tile_pool(name="o2", bufs=1) as o2p,
            tc.tile_pool(name="psS", bufs=1, space="PSUM") as psS,
            tc.tile_pool(name="scr", bufs=1) as scr,
        ):
            for tb in TB:
                IN = io.tile([128, 12 * YP], BF16, tag="IN")
                MK = mki.tile([128, 3 * YP], U8, tag="MK")
                OUT2 = o2p.tile([128, 3 * NY], BF16, tag="OUT2")
                inv12 = IN[:].rearrange("p (c y) -> p c y", c=12)
                nc.sync.dma_start(
                    out=inv12[:, 9:12, :],
                    in_=fin_d[9:12, tb:tb + 128, :].rearrange("c p y -> p c y"))
                nc.sync.dma_start(
                    out=inv12[:, 0:9, :],
                    in_=fin_d[0:9, tb:tb + 128, :].rearrange("c p y -> p c y"))
                nc.sync.dma_start(
                    out=MK[:].rearrange("p (c y) -> p c y", c=3),
                    in_=mk_d[:, tb:tb + 128, :].rearrange("c p y -> p c y"))

                def F(i, a=0, b=YP):
                    return IN[:, i * YP + a:i * YP + b]
                UX = F(9); UY = F(10); RH = F(11)
                def MKV(s, a, b):
                    return MK[:, (s + 1) * YP + a:(s + 1) * YP + b]

                def S(name, dt=BF16, wdt=YP):
                    return scr.tile([128, wdt], dt, tag=name, name=name)

                r1 = S("r1"); r2 = S("r2"); t1 = S("t1"); t2 = S("t2")
                sv = S("sv"); dv = S("dv"); rs = S("rs"); rd = S("rd")
                a5 = S("a5"); a6 = S("a6"); uq = S("uq")
                V = nc.vector; P = nc.gpsimd

                tt(P, r1[:], RH, UX, AL.mult)
                tt(P, r2[:], RH, UY, AL.mult)
                tt(P, t1[:], UX, r1[:], AL.mult)
                tt(P, t2[:], UY, r2[:], AL.mult)
                tt(V, uq[:], t1[:], t2[:], AL.add)
                V.tensor_scalar_mul(uq[:], uq[:], -1.5)
                tt(V, uq[:], uq[:], RH, AL.add)
                tt(P, sv[:], UX, UY, AL.add)
                tt(P, dv[:], UX, UY, AL.subtract)
                tt(V, rs[:], r1[:], r2[:], AL.add)
                tt(V, rd[:], r1[:], r2[:], AL.subtract)
                tt(P, a5[:], sv[:], rs[:], AL.mult)
                tt(P, a6[:], dv[:], rd[:], AL.mult)
                V.tensor_scalar_mul(t1[:], t1[:], 4.5 * W1P)
                V.tensor_scalar_mul(t2[:], t2[:], 4.5 * W1P)
                V.tensor_scalar_mul(a5[:], a5[:], 4.5 * W5P)
                V.tensor_scalar_mul(a6[:], a6[:], 4.5 * W5P)
                V.tensor_scalar_mul(r1[:], r1[:], 3.0 * W1P)
                V.tensor_scalar_mul(r2[:], r2[:], 3.0 * W1P)
                V.tensor_scalar_mul(rs[:], rs[:], 3.0 * W5P)
                V.tensor_scalar_mul(rd[:], rd[:], 3.0 * W5P)
                V.tensor_scalar_mul(sv[:], uq[:], W1P)
                V.tensor_scalar_mul(dv[:], uq[:], W5P)
                V.tensor_scalar_mul(uq[:], uq[:], W0P)
                tt(P, t1[:], t1[:], sv[:], AL.add)
                tt(P, t2[:], t2[:], sv[:], AL.add)
                tt(P, a5[:], a5[:], dv[:], AL.add)
                tt(P, a6[:], a6[:], dv[:], AL.add)
                tt(V, F(0), F(0), uq[:], AL.add)
                tt(V, F(1), F(1), t1[:], AL.add)
                tt(V, F(1), F(1), r1[:], AL.add)
                tt(V, F(3), F(3), t1[:], AL.add)
                tt(V, F(3), F(3), r1[:], AL.subtract)
                tt(V, F(2), F(2), t2[:], AL.add)
                tt(V, F(2), F(2), r2[:], AL.add)
                tt(P, F(4), F(4), t2[:], AL.add)
                tt(P, F(4), F(4), r2[:], AL.subtract)
                tt(P, F(5), F(5), a5[:], AL.add)
                tt(P, F(5), F(5), rs[:], AL.add)
                tt(P, F(7), F(7), a5[:], AL.add)
                tt(P, F(7), F(7), rs[:], AL.subtract)
                tt(P, F(6), F(6), a6[:], AL.add)
                tt(P, F(6), F(6), rd[:], AL.subtract)
                tt(P, F(8), F(8), a6[:], AL.add)
                tt(P, F(8), F(8), rd[:], AL.add)

                nc.sync.dma_start(
                    out=out_d[0:1, tb:tb + 128, :].rearrange("c p y -> p c y"),
                    in_=inv12[:, 0:1, 1:1 + NY])

                for i, j in ((1, 3), (2, 4), (5, 7), (6, 8)):
                    bbs = {}
                    for d, tag in ((i, "bb0"), (j, "bb1")):
                        exd, eyd = EX[d], EY[d]
                        pa, ma, oa = _pamaoa(eyd)
                        bb = S(tag, wdt=NY)
                        for c in range(NCH):
                            sp = psS.tile([128, W], FP32, tag=f"sp{c % 4}",
                                          name=f"sp{d}_{c}")
                            nc.tensor.matmul(sp[:], wblk(exd),
                                             F(OPP[d], ma + W * c, ma + W * (c + 1)))
                            nc.scalar.copy(bb[:, W * c:W * (c + 1)], sp[:])
                        bbs[d] = bb
                    for d in (i, j):
                        exd, eyd = EX[d], EY[d]
                        pa, ma, oa = _pamaoa(eyd)
                        V.copy_predicated(F(d, oa, oa + NY),
                                          MKV(exd, pa, pa + NY), bbs[d][:])
                    for d in (i, j):
                        if EY[d] == 1:
                            nc.scalar.copy(F(d, 0, 1), F(d, NY, NY + 1))
                        elif EY[d] == -1:
                            nc.scalar.copy(F(d, NY + 1, NY + 2), F(d, 1, 2))
                    nc.sync.dma_start(
                        out=out_d[i:i + 1, tb:tb + 128, :].rearrange("c p y -> p c y"),
                        in_=inv12[:, i:i + 1, 1:1 + NY])
                    nc.sync.dma_start(
                        out=out_d[j:j + 1, tb:tb + 128, :].rearrange("c p y -> p c y"),
                        in_=inv12[:, j:j + 1, 1:1 + NY])

                sm1 = S("sv", wdt=NY); sm2 = S("dv", wdt=NY)
                inv = S("inv", FP32, wdt=NY)
                for c in range(NCH):
                    cs = slice(W * c, W * (c + 1))
                    def mv(i):
                        a = 1 - EY[i] + W * c
                        return F(i, a, a + W)
                    rp = psS.tile([128, W], FP32, tag=f"sp{c % 2}", name=f"rp{c}")
                    for k, i in enumerate(range(9)):
                        nc.tensor.matmul(rp[:], wblk(-EX[i]), mv(i),
                                         start=(k == 0), stop=(k == 8))
                    m1p = psS.tile([128, W], FP32, tag=f"sp{2 + c % 2}", name=f"m1p{c}")
                    for k, i in enumerate((1, 5, 8, 3, 6, 7)):
                        nc.tensor.matmul(m1p[:], wblk(-EX[i], neg=(EX[i] < 0)), mv(i),
                                         start=(k == 0), stop=(k == 5))
                    m2p = psS.tile([128, W], FP32, tag=f"sp{4 + c % 2}", name=f"m2p{c}")
                    for k, i in enumerate((2, 5, 6, 4, 7, 8)):
                        nc.tensor.matmul(m2p[:], wblk(-EX[i], neg=(EY[i] < 0)), mv(i),
                                         start=(k == 0), stop=(k == 5))
                    nc.scalar.copy(OUT2[:, cs], rp[:])
                    V.reciprocal_approx_fast(inv[:, cs], rp[:])
                    nc.scalar.copy(sm1[:, cs], m1p[:])
                    nc.scalar.copy(sm2[:, cs], m2p[:])
                invb = S("invb", wdt=NY)
                nc.scalar.copy(invb[:], inv[:])
                tt(V, OUT2[:, NY:2 * NY], sm1[:], invb[:], AL.mult)
                tt(V, OUT2[:, 2 * NY:3 * NY], sm2[:], invb[:], AL.mult)

                nc.sync.dma_start(
                    out=out_d[9:12, tb + 1:tb + 127, :].rearrange("c p y -> p c y"),
                    in_=OUT2[1:127, :].rearrange("p (c y) -> p c y", c=3))

            pIN = mki.tile([FXP, 12 * FXF], BF16, tag="pIN")
            pMK = mki.tile([FXP, 3 * FXF], U8, tag="pMK")
            nc.sync.dma_start(
                out=pIN[:].rearrange("p (c y) -> p c y", c=12),
                in_=pfin_d[:, :, :].rearrange("c p y -> p c y"))
            nc.sync.dma_start(
                out=pMK[:].rearrange("p (c y) -> p c y", c=3),
                in_=pmk_d[:, :, :].rearrange("c p y -> p c y"))

            def pF(i, a=0, b=FXF):
                return pIN[:, i * FXF + a:i * FXF + b]
            pUX = pF(9); pUY = pF(10); pRH = pF(11)
            def pMKV(s, a, b):
                return pMK[:, (s + 1) * FXF + a:(s + 1) * FXF + b]

            def PS(name, dt=BF16, wdt=FXF):
                return scr.tile([FXP, wdt], dt, tag=f"p_{name}", name=f"p_{name}")

            r1 = PS("r1"); r2 = PS("r2"); t1 = PS("t1"); t2 = PS("t2")
            sv = PS("sv"); dv = PS("dv"); rs = PS("rs"); rd = PS("rd")
            a5 = PS("a5"); a6 = PS("a6"); uq = PS("uq")
            V = nc.vector; P = nc.gpsimd

            tt(P, r1[:], pRH, pUX, AL.mult)
            tt(P, r2[:], pRH, pUY, AL.mult)
            tt(P, t1[:], pUX, r1[:], AL.mult)
            tt(P, t2[:], pUY, r2[:], AL.mult)
            tt(V, uq[:], t1[:], t2[:], AL.add)
            V.tensor_scalar_mul(uq[:], uq[:], -1.5)
            tt(V, uq[:], uq[:], pRH, AL.add)
            tt(P, sv[:], pUX, pUY, AL.add)
            tt(P, dv[:], pUX, pUY, AL.subtract)
            tt(V, rs[:], r1[:], r2[:], AL.add)
            tt(V, rd[:], r1[:], r2[:], AL.subtract)
            tt(P, a5[:], sv[:], rs[:], AL.mult)
            tt(P, a6[:], dv[:], rd[:], AL.mult)
            V.tensor_scalar_mul(t1[:], t1[:], 4.5 * W1P)
            V.tensor_scalar_mul(t2[:], t2[:], 4.5 * W1P)
            V.tensor_scalar_mul(a5[:], a5[:], 4.5 * W5P)
            V.tensor_scalar_mul(a6[:], a6[:], 4.5 * W5P)
            V.tensor_scalar_mul(r1[:], r1[:], 3.0 * W1P)
            V.tensor_scalar_mul(r2[:], r2[:], 3.0 * W1P)
            V.tensor_scalar_mul(rs[:], rs[:], 3.0 * W5P)
            V.tensor_scalar_mul(rd[:], rd[:], 3.0 * W5P)
            V.tensor_scalar_mul(sv[:], uq[:], W1P)
            V.tensor_scalar_mul(dv[:], uq[:], W5P)
            V.tensor_scalar_mul(uq[:], uq[:], W0P)
            tt(P, t1[:], t1[:], sv[:], AL.add)
            tt(P, t2[:], t2[:], sv[:], AL.add)
            tt(P, a5[:], a5[:], dv[:], AL.add)
            tt(P, a6[:], a6[:], dv[:], AL.add)
            tt(V, pF(0), pF(0), uq[:], AL.add)
            tt(V, pF(1), pF(1), t1[:], AL.add)
            tt(V, pF(1), pF(1), r1[:], AL.add)
            tt(V, pF(3), pF(3), t1[:], AL.add)
            tt(V, pF(3), pF(3), r1[:], AL.subtract)
            tt(V, pF(2), pF(2), t2[:], AL.add)
            tt(V, pF(2), pF(2), r2[:], AL.add)
            tt(P, pF(4), pF(4), t2[:], AL.add)
            tt(P, pF(4), pF(4), r2[:], AL.subtract)
            tt(P, pF(5), pF(5), a5[:], AL.add)
            tt(P, pF(5), pF(5), rs[:], AL.add)
            tt(P, pF(7), pF(7), a5[:], AL.add)
            tt(P, pF(7), pF(7), rs[:], AL.subtract)
            tt(P, pF(6), pF(6), a6[:], AL.add)
            tt(P, pF(6), pF(6), rd[:], AL.subtract)
            tt(P, pF(8), pF(8), a6[:], AL.add)
            tt(P, pF(8), pF(8), rd[:], AL.add)

            for i, j in ((1, 3), (2, 4), (5, 7), (6, 8)):
                bbs = {}
                for d, tag in ((i, "bb0"), (j, "bb1")):
                    exd, eyd = EX[d], EY[d]
                    pa, ma, oa = _pamaoa(eyd)
                    bb = PS(tag, wdt=FXW + 2)
                    sp = psS.tile([FXP, FXW + 2], FP32, tag=f"sp{0 if d == i else 1}",
                                  name=f"psp{d}")
                    nc.tensor.matmul(sp[:], pwblk(exd), pF(OPP[d], ma, ma + FXW + 2))
                    nc.scalar.copy(bb[:], sp[:])
                    bbs[d] = bb
                for d in (i, j):
                    exd, eyd = EX[d], EY[d]
                    pa, ma, oa = _pamaoa(eyd)
                    V.copy_predicated(pF(d, oa, oa + FXW + 2),
                                      pMKV(exd, pa, pa + FXW + 2), bbs[d][:])

            def pmv(i):
                a = 2 - EY[i]
                return pF(i, a, a + FXW)
            rp = psS.tile([FXP, FXW], FP32, tag="sp0", name="prp")
            for k, i in enumerate(range(9)):
                nc.tensor.matmul(rp[:], pwblk(-EX[i]), pmv(i),
                                 start=(k == 0), stop=(k == 8))
            m1p = psS.tile([FXP, FXW], FP32, tag="sp2", name="pm1p")
            for k, i in enumerate((1, 5, 8, 3, 6, 7)):
                nc.tensor.matmul(m1p[:], pwblk(-EX[i], neg=(EX[i] < 0)), pmv(i),
                                 start=(k == 0), stop=(k == 5))
            m2p = psS.tile([FXP, FXW], FP32, tag="sp4", name="pm2p")
            for k, i in enumerate((2, 5, 6, 4, 7, 8)):
                nc.tensor.matmul(m2p[:], pwblk(-EX[i], neg=(EY[i] < 0)), pmv(i),
                                 start=(k == 0), stop=(k == 5))
            pinv = PS("pinv", FP32, wdt=FXW)
            psm1 = PS("sv", wdt=FXW); psm2 = PS("dv", wdt=FXW)
            nc.scalar.copy(pF(9, 2, 2 + FXW), rp[:])
            V.reciprocal_approx_fast(pinv[:], rp[:])
            nc.scalar.copy(psm1[:], m1p[:])
            nc.scalar.copy(psm2[:], m2p[:])
            pinvb = PS("pinvb", wdt=FXW)
            nc.scalar.copy(pinvb[:], pinv[:])
            tt(V, pF(10, 2, 2 + FXW), psm1[:], pinvb[:], AL.mult)
            tt(V, pF(11, 2, 2 + FXW), psm2[:], pinvb[:], AL.mult)

            for sg in range(FXSEG):
                nc.sync.dma_start(
                    out=out_d[:, 127:131, sg * FXW:(sg + 1) * FXW].rearrange(
                        "c r y -> r c y"),
                    in_=pIN[sg * FXNR + 1:sg * FXNR + 5, :].rearrange(
                        "p (c y) -> p c y", c=12)[:, :, 2:2 + FXW])

    nc.finalize()
    return nc


_NC_CACHE = None


def _get_nc():
    global _NC_CACHE
    if _NC_CACHE is None:
        _NC_CACHE = _build_program()
    return _NC_CACHE


def _wts_np():
    import ml_dtypes
    m = np.zeros((128, 6 * 128), np.float32)
    for bi, (s, sgn) in enumerate([(-1, 1), (0, 1), (1, 1), (-1, -1), (0, -1), (1, -1)]):
        for q in range(128):
            k = q + s
            if 0 <= k < 128:
                m[k, bi * 128 + q] = sgn
    return m.astype(ml_dtypes.bfloat16)


def _pwts_np():
    import ml_dtypes
    m = np.zeros((FXP, 6 * FXP), np.float32)
    for bi, (s, sgn) in enumerate([(-1, 1), (0, 1), (1, 1), (-1, -1), (0, -1), (1, -1)]):
        for sg in range(FXSEG):
            for j in range(FXNR):
                q = sg * FXNR + j
                jk = j + s
                if 0 <= jk < FXNR:
                    m[sg * FXNR + jk, bi * FXP + q] = sgn
    return m.astype(ml_dtypes.bfloat16)


def _prep_inputs(f, rho, u, obstacle_mask):
    import ml_dtypes
    f = np.asarray(f, dtype=np.float32)
    rho = np.asarray(rho, dtype=np.float32)
    u = np.asarray(u, dtype=np.float32)
    mask = np.asarray(obstacle_mask).astype(np.uint8)

    planes = np.empty((12, NX, NY), np.float32)
    for i in range(9):
        planes[i] = FCOEF * f[..., i]
    planes[9] = u[..., 0]
    planes[10] = u[..., 1]
    planes[11] = rho
    planes_b = planes.astype(ml_dtypes.bfloat16)

    wts = _wts_np()
    pwts = _pwts_np()
    rows_idx = np.arange(-1, R + 1)
    cols_idx = np.arange(-1, NY + 1) % NY
    in_maps = []
    for k in range(NCORES):
        lo = k * R
        ridx = (lo + rows_idx) % NX
        fin = planes_b[:, ridx][:, :, cols_idx]
        mk = np.empty((3, SLAB, YP), np.uint8)
        for si, s in enumerate((-1, 0, 1)):
            mk[si] = mask[(lo + rows_idx + s) % NX][:, cols_idx]
        pfin = np.empty((12, FXP, FXF), ml_dtypes.bfloat16)
        pmk = np.empty((3, FXP, FXF), np.uint8)
        frows = (lo - 1 + FXR0 + np.arange(FXNR)) % NX
        for sg in range(FXSEG):
            ccols = (sg * FXW + np.arange(-2, FXW + 2)) % NY
            seg = planes_b[:, frows][:, :, ccols]
            pfin[:, sg * FXNR:(sg + 1) * FXNR] = seg
            for si, s in enumerate((-1, 0, 1)):
                pmk[si, sg * FXNR:(sg + 1) * FXNR] = \
                    mask[(frows + s) % NX][:, ccols]
        in_maps.append({
            "fin": np.ascontiguousarray(fin),
            "mk": np.ascontiguousarray(mk),
            "wts": wts,
            "pfin": np.ascontiguousarray(pfin),
            "pmk": np.ascontiguousarray(pmk),
            "pwts": pwts,
        })
    return in_maps


def kernel(f, rho, u, obstacle_mask, _trace=False):
    in_maps = _prep_inputs(f, rho, u, obstacle_mask)
    nc = _get_nc()
    res = run_bass_kernel_spmd(nc, in_maps, list(range(NCORES)),
                               trace=bool(_trace))
    full = np.empty((NX, NY, 12), np.float32)
    chan = np.concatenate(
        [np.asarray(res.results[k]["out"])[:, 1:R + 1, :].astype(np.float32)
         for k in range(NCORES)], axis=1)
    for c in range(9):
        full[..., c] = np.roll(chan[c], (EX[c], EY[c]), axis=(0, 1))
    full[..., 9] = chan[9]
    full[..., 10] = chan[10]
    full[..., 11] = chan[11]
    if _trace:
        return full, res
    return full


# revision 18
# speedup vs baseline: 1.2421x; 1.1216x over previous
import numpy as np
import concourse.bass as bass
import concourse.bacc as bacc
import concourse.mybir as mybir
from concourse import tile
from concourse.bass_utils import run_bass_kernel_spmd

NX = 2048
NY = 2048
NCORES = 8
R = NX // NCORES
SLAB = R + 2
YP = NY + 2
TB = [0, 130]
W = 512
NCH = NY // W

TAU = 0.6
INV_TAU = 1.0 / TAU
FCOEF = 1.0 - INV_TAU
W1P = INV_TAU * (1.0 / 9.0)
W5P = INV_TAU * (1.0 / 36.0)
W0P = INV_TAU * (4.0 / 9.0)

EX = [0, 1, 0, -1, 0, 1, -1, -1, 1]
EY = [0, 0, 1, 0, -1, 1, 1, -1, -1]
OPP = [0, 3, 4, 1, 2, 7, 8, 5, 6]

FXR0 = 126
FXNR = 6
FXSEG = 8
FXW = NY // FXSEG
FXF = FXW + 4
FXP = FXSEG * FXNR

FP32 = mybir.dt.float32
BF16 = mybir.dt.bfloat16
U8 = mybir.dt.uint8
AL = mybir.AluOpType

def _pamaoa(ey):
    pa = 1 + ey
    return pa, pa, 1


def _build_program():
    nc = bacc.Bacc(None)

    fin_d = nc.declare_dram_parameter("fin", [12, SLAB, YP], BF16, isOutput=False)
    mk_d = nc.declare_dram_parameter("mk", [3, SLAB, YP], U8, isOutput=False)
    wts_d = nc.declare_dram_parameter("wts", [128, 6 * 128], BF16, isOutput=False)
    pfin_d = nc.declare_dram_parameter("pfin", [12, FXP, FXF], BF16, isOutput=False)
    pmk_d = nc.declare_dram_parameter("pmk", [3, FXP, FXF], U8, isOutput=False)
    pwts_d = nc.declare_dram_parameter("pwts", [FXP, 6 * FXP], BF16, isOutput=False)
    out_d = nc.declare_dram_parameter("out", [12, SLAB, NY], BF16, isOutput=True)

    def tt(eng, o, a, b, op):
        eng.tensor_tensor(o, a, b, op)

    with tile.TileContext(nc) as tc, tc.tile_pool(name="cst", bufs=1) as cst:
        wts = cst.tile([128, 6 * 128], BF16)
        pwts = cst.tile([FXP, 6 * FXP], BF16)
        nc.sync.dma_start(out=wts[:], in_=wts_d[:, :])
        nc.sync.dma_start(out=pwts[:], in_=pwts_d[:, :])
        def wblk(s, neg=False):
            b = (s + 1) + (3 if neg else 0)
            return wts[:, b * 128:(b + 1) * 128]
        def pwblk(s, neg=False):
            b = (s + 1) + (3 if neg else 0)
            return pwts[:, b * FXP:(b + 1) * FXP]

        with (
            tc.tile_pool(name="io", bufs=2) as io,
            tc.tile_pool(name="mki", bufs=1) as mki,
            tc.tile_pool(name="o2", bufs=1) as o2p,
            tc.tile_pool(name="psS", bufs=1, space="PSUM") as psS,
            tc.tile_pool(name="scr", bufs=1) as scr,
        ):
            for tb in TB:
                IN = io.tile([128, 12 * YP], BF16, tag="IN")
                MK = mki.tile([128, 3 * YP], U8, tag="MK")
                OUT2 = o2p.tile([128, 3 * NY], BF16, tag="OUT2")
                inv12 = IN[:].rearrange("p (c y) -> p c y", c=12)
                nc.sync.dma_start(
                    out=inv12[:, 9:12, :],
                    in_=fin_d[9:12, tb:tb + 128, :].rearrange("c p y -> p c y"))
                nc.sync.dma_start(
                    out=inv12[:, 0:9, :],
                    in_=fin_d[0:9, tb:tb + 128, :].rearrange("c p y -> p c y"))
                nc.sync.dma_start(
                    out=MK[:].rearrange("p (c y) -> p c y", c=3),
                    in_=mk_d[:, tb:tb + 128, :].rearrange("c p y -> p c y"))

                def F(i, a=0, b=YP):
                    return IN[:, i * YP + a:i * YP + b]
                UX = F(9); UY = F(10); RH = F(11)
                def MKV(s, a, b):
                    return MK[:, (s + 1) * YP + a:(s + 1) * YP + b]

                def S(name, dt=BF16, wdt=YP):
                    return scr.tile([128, wdt], dt, tag=name, name=name)

                r1 = S("r1"); r2 = S("r2"); t1 = S("t1"); t2 = S("t2")
                sv = S("sv"); dv = S("dv"); rs = S("rs"); rd = S("rd")
                a5 = S("a5"); a6 = S("a6"); uq = S("uq")
                V = nc.vector; P = nc.gpsimd

                tt(P, r1[:], RH, UX, AL.mult)
                tt(P, r2[:], RH, UY, AL.mult)
                tt(P, t1[:], UX, r1[:], AL.mult)
                tt(P, t2[:], UY, r2[:], AL.mult)
                tt(V, uq[:], t1[:], t2[:], AL.add)
                V.tensor_scalar_mul(uq[:], uq[:], -1.5)
                tt(V, uq[:], uq[:], RH, AL.add)
                tt(P, sv[:], UX, UY, AL.add)
                tt(P, dv[:], UX, UY, AL.subtract)
                tt(P, rs[:], r1[:], r2[:], AL.add)
                tt(P, rd[:], r1[:], r2[:], AL.subtract)
                tt(P, a5[:], sv[:], rs[:], AL.mult)
                tt(P, a6[:], dv[:], rd[:], AL.mult)
                V.tensor_scalar_mul(t1[:], t1[:], 4.5 * W1P)
                V.tensor_scalar_mul(t2[:], t2[:], 4.5 * W1P)
                V.tensor_scalar_mul(a5[:], a5[:], 4.5 * W5P)
                V.tensor_scalar_mul(a6[:], a6[:], 4.5 * W5P)
                V.tensor_scalar_mul(r1[:], r1[:], 3.0 * W1P)
                V.tensor_scalar_mul(r2[:], r2[:], 3.0 * W1P)
                nc.scalar.mul(rs[:], rs[:], 3.0 * W5P)
                nc.scalar.mul(rd[:], rd[:], 3.0 * W5P)
                V.tensor_scalar_mul(sv[:], uq[:], W1P)
                V.tensor_scalar_mul(dv[:], uq[:], W5P)
                V.tensor_scalar_mul(uq[:], uq[:], W0P)
                tt(P, t1[:], t1[:], sv[:], AL.add)
                tt(P, t2[:], t2[:], sv[:], AL.add)
                tt(P, a5[:], a5[:], dv[:], AL.add)
                tt(P, a6[:], a6[:], dv[:], AL.add)
                tt(V, F(0), F(0), uq[:], AL.add)
                tt(V, F(1), F(1), t1[:], AL.add)
                tt(V, F(1), F(1), r1[:], AL.add)
                tt(V, F(3), F(3), t1[:], AL.add)
                tt(V, F(3), F(3), r1[:], AL.subtract)
                tt(V, F(2), F(2), t2[:], AL.add)
                tt(V, F(2), F(2), r2[:], AL.add)
                tt(P, F(4), F(4), t2[:], AL.add)
                tt(P, F(4), F(4), r2[:], AL.subtract)
                tt(P, F(5), F(5), a5[:], AL.add)
                tt(P, F(5), F(5), rs[:], AL.add)
                tt(P, F(7), F(7), a5[:], AL.add)
                tt(P, F(7), F(7), rs[:], AL.subtract)
                tt(P, F(6), F(6), a6[:], AL.add)
                tt(P, F(6), F(6), rd[:], AL.subtract)
                tt(P, F(8), F(8), a6[:], AL.add)
                tt(P, F(8), F(8), rd[:], AL.add)

                nc.sync.dma_start(
                    out=out_d[0:1, tb:tb + 128, :].rearrange("c p y -> p c y"),
                    in_=inv12[:, 0:1, 1:1 + NY])

                for i, j in ((1, 3), (2, 4), (5, 7), (6, 8)):
                    bbs = {}
                    for d, tag in ((i, "bb0"), (j, "bb1")):
                        exd, eyd = EX[d], EY[d]
                        pa, ma, oa = _pamaoa(eyd)
                        bb = S(tag, wdt=NY)
                        for c in range(NCH):
                            sp = psS.tile([128, W], FP32, tag=f"sp{c % 4}",
                                          name=f"sp{d}_{c}")
                            nc.tensor.matmul(sp[:], wblk(exd),
                                             F(OPP[d], ma + W * c, ma + W * (c + 1)))
                            nc.scalar.copy(bb[:, W * c:W * (c + 1)], sp[:])
                        bbs[d] = bb
                    for d in (i, j):
                        exd, eyd = EX[d], EY[d]
                        pa, ma, oa = _pamaoa(eyd)
                        V.copy_predicated(F(d, oa, oa + NY),
                                          MKV(exd, pa, pa + NY), bbs[d][:])
                    for d in (i, j):
                        if EY[d] == 1:
                            nc.scalar.copy(F(d, 0, 1), F(d, NY, NY + 1))
                        elif EY[d] == -1:
                            nc.scalar.copy(F(d, NY + 1, NY + 2), F(d, 1, 2))
                    nc.sync.dma_start(
                        out=out_d[i:i + 1, tb:tb + 128, :].rearrange("c p y -> p c y"),
                        in_=inv12[:, i:i + 1, 1:1 + NY])
                    nc.sync.dma_start(
                        out=out_d[j:j + 1, tb:tb + 128, :].rearrange("c p y -> p c y"),
                        in_=inv12[:, j:j + 1, 1:1 + NY])

                sm1 = S("sv", wdt=NY); sm2 = S("dv", wdt=NY)
                inv = S("inv", FP32, wdt=NY)
                def mv(i, c):
                    a = 1 - EY[i] + W * c
                    return F(i, a, a + W)
                rps = []
                for c in range(NCH):
                    rp = psS.tile([128, W], FP32, tag=f"sp{c % 2}", name=f"rp{c}")
                    for k, i in enumerate(range(9)):
                        nc.tensor.matmul(rp[:], wblk(-EX[i]), mv(i, c),
                                         start=(k == 0), stop=(k == 8))
                    cs = slice(W * c, W * (c + 1))
                    nc.scalar.copy(OUT2[:, cs], rp[:])
                    V.reciprocal_approx_fast(inv[:, cs], rp[:])
                for c in range(NCH):
                    cs = slice(W * c, W * (c + 1))
                    m1p = psS.tile([128, W], FP32, tag=f"sp{2 + c % 2}", name=f"m1p{c}")
                    for k, i in enumerate((1, 5, 8, 3, 6, 7)):
                        nc.tensor.matmul(m1p[:], wblk(-EX[i], neg=(EX[i] < 0)), mv(i, c),
                                         start=(k == 0), stop=(k == 5))
                    nc.scalar.copy(sm1[:, cs], m1p[:])
                for c in range(NCH):
                    cs = slice(W * c, W * (c + 1))
                    m2p = psS.tile([128, W], FP32, tag=f"sp{4 + c % 2}", name=f"m2p{c}")
                    for k, i in enumerate((2, 5, 6, 4, 7, 8)):
                        nc.tensor.matmul(m2p[:], wblk(-EX[i], neg=(EY[i] < 0)), mv(i, c),
                                         start=(k == 0), stop=(k == 5))
                    nc.scalar.copy(sm2[:, cs], m2p[:])
                invb = S("invb", wdt=NY)
                nc.scalar.copy(invb[:], inv[:])
                tt(V, OUT2[:, NY:2 * NY], sm1[:], invb[:], AL.mult)
                tt(V, OUT2[:, 2 * NY:3 * NY], sm2[:], invb[:], AL.mult)

                nc.sync.dma_start(
                    out=out_d[9:12, tb + 1:tb + 127, :].rearrange("c p y -> p c y"),
                    in_=OUT2[1:127, :].rearrange("p (c y) -> p c y", c=3))

            pIN = mki.tile([FXP, 12 * FXF], BF16, tag="pIN")
            pMK = mki.tile([FXP, 3 * FXF], U8, tag="pMK")
            nc.sync.dma_start(
                out=pIN[:].rearrange("p (c y) -> p c y", c=12),
                in_=pfin_d[:, :, :].rearrange("c p y -> p c y"))
            nc.sync.dma_start(
                out=pMK[:].rearrange("p (c y) -> p c y", c=3),
                in_=pmk_d[:, :, :].rearrange("c p y -> p c y"))

            def pF(i, a=0, b=FXF):
                return pIN[:, i * FXF + a:i * FXF + b]
            pUX = pF(9); pUY = pF(10); pRH = pF(11)
            def pMKV(s, a, b):
                return pMK[:, (s + 1) * FXF + a:(s + 1) * FXF + b]

            def PS(name, dt=BF16, wdt=FXF):
                return scr.tile([FXP, wdt], dt, tag=f"p_{name}", name=f"p_{name}")

            r1 = PS("r1"); r2 = PS("r2"); t1 = PS("t1"); t2 = PS("t2")
            sv = PS("sv"); dv = PS("dv"); rs = PS("rs"); rd = PS("rd")
            a5 = PS("a5"); a6 = PS("a6"); uq = PS("uq")
            V = nc.vector; P = nc.gpsimd

            tt(P, r1[:], pRH, pUX, AL.mult)
            tt(P, r2[:], pRH, pUY, AL.mult)
            tt(P, t1[:], pUX, r1[:], AL.mult)
            tt(P, t2[:], pUY, r2[:], AL.mult)
            tt(V, uq[:], t1[:], t2[:], AL.add)
            V.tensor_scalar_mul(uq[:], uq[:], -1.5)
            tt(V, uq[:], uq[:], pRH, AL.add)
            tt(P, sv[:], pUX, pUY, AL.add)
            tt(P, dv[:], pUX, pUY, AL.subtract)
            tt(V, rs[:], r1[:], r2[:], AL.add)
            tt(V, rd[:], r1[:], r2[:], AL.subtract)
            tt(P, a5[:], sv[:], rs[:], AL.mult)
            tt(P, a6[:], dv[:], rd[:], AL.mult)
            V.tensor_scalar_mul(t1[:], t1[:], 4.5 * W1P)
            V.tensor_scalar_mul(t2[:], t2[:], 4.5 * W1P)
            V.tensor_scalar_mul(a5[:], a5[:], 4.5 * W5P)
            V.tensor_scalar_mul(a6[:], a6[:], 4.5 * W5P)
            V.tensor_scalar_mul(r1[:], r1[:], 3.0 * W1P)
            V.tensor_scalar_mul(r2[:], r2[:], 3.0 * W1P)
            V.tensor_scalar_mul(rs[:], rs[:], 3.0 * W5P)
            V.tensor_scalar_mul(rd[:], rd[:], 3.0 * W5P)
            V.tensor_scalar_mul(sv[:], uq[:], W1P)
            V.tensor_scalar_mul(dv[:], uq[:], W5P)
            V.tensor_scalar_mul(uq[:], uq[:], W0P)
            tt(P, t1[:], t1[:], sv[:], AL.add)
            tt(P, t2[:], t2[:], sv[:], AL.add)
            tt(P, a5[:], a5[:], dv[:], AL.add)
            tt(P, a6[:], a6[:], dv[:], AL.add)
            tt(V, pF(0), pF(0), uq[:], AL.add)
            tt(V, pF(1), pF(1), t1[:], AL.add)
            tt(V, pF(1), pF(1), r1[:], AL.add)
            tt(V, pF(3), pF(3), t1[:], AL.add)
            tt(V, pF(3), pF(3), r1[:], AL.subtract)
            tt(V, pF(2), pF(2), t2[:], AL.add)
            tt(V, pF(2), pF(2), r2[:], AL.add)
            tt(P, pF(4), pF(4), t2[:], AL.add)
            tt(P, pF(4), pF(4), r2[:], AL.subtract)
            tt(P, pF(5), pF(5), a5[:], AL.add)
            tt(P, pF(5), pF(5), rs[:], AL.add)
            tt(P, pF(7), pF(7), a5[:], AL.add)
            tt(P, pF(7), pF(7), rs[:], AL.subtract)
            tt(P, pF(6), pF(6), a6[:], AL.add)
            tt(P, pF(6), pF(6), rd[:], AL.subtract)
            tt(P, pF(8), pF(8), a6[:], AL.add)
            tt(P, pF(8), pF(8), rd[:], AL.add)

            for i, j in ((1, 3), (2, 4), (5, 7), (6, 8)):
                bbs = {}
                for d, tag in ((i, "bb0"), (j, "bb1")):
                    exd, eyd = EX[d], EY[d]
                    pa, ma, oa = _pamaoa(eyd)
                    bb = PS(tag, wdt=FXW + 2)
                    sp = psS.tile([FXP, FXW + 2], FP32, tag=f"sp{0 if d == i else 1}",
                                  name=f"psp{d}")
                    nc.tensor.matmul(sp[:], pwblk(exd), pF(OPP[d], ma, ma + FXW + 2))
                    nc.scalar.copy(bb[:], sp[:])
                    bbs[d] = bb
                for d in (i, j):
                    exd, eyd = EX[d], EY[d]
                    pa, ma, oa = _pamaoa(eyd)
                    V.copy_predicated(pF(d, oa, oa + FXW + 2),
                                      pMKV(exd, pa, pa + FXW + 2), bbs[d][:])

            def pmv(i):
                a = 2 - EY[i]
                return pF(i, a, a + FXW)
            rp = psS.tile([FXP, FXW], FP32, tag="sp0", name="prp")
            for k, i in enumerate(range(9)):
                nc.tensor.matmul(rp[:], pwblk(-EX[i]), pmv(i),
                                 start=(k == 0), stop=(k == 8))
            m1p = psS.tile([FXP, FXW], FP32, tag="sp2", name="pm1p")
            for k, i in enumerate((1, 5, 8, 3, 6, 7)):
                nc.tensor.matmul(m1p[:], pwblk(-EX[i], neg=(EX[i] < 0)), pmv(i),
                                 start=(k == 0), stop=(k == 5))
            m2p = psS.tile([FXP, FXW], FP32, tag="sp4", name="pm2p")
            for k, i in enumerate((2, 5, 6, 4, 7, 8)):
                nc.tensor.matmul(m2p[:], pwblk(-EX[i], neg=(EY[i] < 0)), pmv(i),
                                 start=(k == 0), stop=(k == 5))
            pinv = PS("pinv", FP32, wdt=FXW)
            psm1 = PS("sv", wdt=FXW); psm2 = PS("dv", wdt=FXW)
            nc.scalar.copy(pF(9, 2, 2 + FXW), rp[:])
            V.reciprocal_approx_fast(pinv[:], rp[:])
            nc.scalar.copy(psm1[:], m1p[:])
            nc.scalar.copy(psm2[:], m2p[:])
            pinvb = PS("pinvb", wdt=FXW)
            nc.scalar.copy(pinvb[:], pinv[:])
            tt(V, pF(10, 2, 2 + FXW), psm1[:], pinvb[:], AL.mult)
            tt(V, pF(11, 2, 2 + FXW), psm2[:], pinvb[:], AL.mult)

            for rr, eng in ((0, nc.sync), (1, nc.scalar),
                            (2, nc.gpsimd), (3, nc.sync)):
                eng.dma_start(
                    out=out_d[:, 127 + rr, :].rearrange("c (s y) -> s c y",
                                                        s=FXSEG),
                    in_=pIN[:].rearrange("(s r) (c y) -> r s c y",
                                         r=FXNR, c=12)[1 + rr][:, :, 2:2 + FXW])

    nc.finalize()
    return nc


_NC_CACHE = None


def _get_nc():
    global _NC_CACHE
    if _NC_CACHE is None:
        _NC_CACHE = _build_program()
    return _NC_CACHE


def _wts_np():
    import ml_dtypes
    m = np.zeros((128, 6 * 128), np.float32)
    for bi, (s, sgn) in enumerate([(-1, 1), (0, 1), (1, 1), (-1, -1), (0, -1), (1, -1)]):
        for q in range(128):
            k = q + s
            if 0 <= k < 128:
                m[k, bi * 128 + q] = sgn
    return m.astype(ml_dtypes.bfloat16)


def _pwts_np():
    import ml_dtypes
    m = np.zeros((FXP, 6 * FXP), np.float32)
    for bi, (s, sgn) in enumerate([(-1, 1), (0, 1), (1, 1), (-1, -1), (0, -1), (1, -1)]):
        for sg in range(FXSEG):
            for j in range(FXNR):
                q = sg * FXNR + j
                jk = j + s
                if 0 <= jk < FXNR:
                    m[sg * FXNR + jk, bi * FXP + q] = sgn
    return m.astype(ml_dtypes.bfloat16)


def _prep_inputs(f, rho, u, obstacle_mask):
    import ml_dtypes
    f = np.asarray(f, dtype=np.float32)
    rho = np.asarray(rho, dtype=np.float32)
    u = np.asarray(u, dtype=np.float32)
    mask = np.asarray(obstacle_mask).astype(np.uint8)

    planes = np.empty((12, NX, NY), np.float32)
    for i in range(9):
        planes[i] = FCOEF * f[..., i]
    planes[9] = u[..., 0]
    planes[10] = u[..., 1]
    planes[11] = rho
    planes_b = planes.astype(ml_dtypes.bfloat16)

    wts = _wts_np()
    pwts = _pwts_np()
    rows_idx = np.arange(-1, R + 1)
    cols_idx = np.arange(-1, NY + 1) % NY
    in_maps = []
    for k in range(NCORES):
        lo = k * R
        ridx = (lo + rows_idx) % NX
        fin = planes_b[:, ridx][:, :, cols_idx]
        mk = np.empty((3, SLAB, YP), np.uint8)
        for si, s in enumerate((-1, 0, 1)):
            mk[si] = mask[(lo + rows_idx + s) % NX][:, cols_idx]
        pfin = np.empty((12, FXP, FXF), ml_dtypes.bfloat16)
        pmk = np.empty((3, FXP, FXF), np.uint8)
        frows = (lo - 1 + FXR0 + np.arange(FXNR)) % NX
        for sg in range(FXSEG):
            ccols = (sg * FXW + np.arange(-2, FXW + 2)) % NY
            seg = planes_b[:, frows][:, :, ccols]
            pfin[:, sg * FXNR:(sg + 1) * FXNR] = seg
            for si, s in enumerate((-1, 0, 1)):
                pmk[si, sg * FXNR:(sg + 1) * FXNR] = \
                    mask[(frows + s) % NX][:, ccols]
        in_maps.append({
            "fin": np.ascontiguousarray(fin),
            "mk": np.ascontiguousarray(mk),
            "wts": wts,
            "pfin": np.ascontiguousarray(pfin),
            "pmk": np.ascontiguousarray(pmk),
            "pwts": pwts,
        })
    return in_maps


def kernel(f, rho, u, obstacle_mask, _trace=False):
    in_maps = _prep_inputs(f, rho, u, obstacle_mask)
    nc = _get_nc()
    res = run_bass_kernel_spmd(nc, in_maps, list(range(NCORES)),
                               trace=bool(_trace))
    full = np.empty((NX, NY, 12), np.float32)
    chan = np.concatenate(
        [np.asarray(res.results[k]["out"])[:, 1:R + 1, :].astype(np.float32)
         for k in range(NCORES)], axis=1)
    for c in range(9):
        full[..., c] = np.roll(chan[c], (EX[c], EY[c]), axis=(0, 1))
    full[..., 9] = chan[9]
    full[..., 10] = chan[10]
    full[..., 11] = chan[11]
    if _trace:
        return full, res
    return full
